# revision 5
# baseline (speedup 1.0000x reference)
"""Trainium2 Bass kernel for GQA causal attention with packed-sequence segment
masking (B=2, T=2048, 32 q-heads, 8 kv-heads, D=128, fp32).

Sharding: 8 NeuronCores; core i handles kv-head i (and its 4 GQA query heads)
for both batch rows. One SPMD program; per-core data differs only in which
heads it gets. The segment layout (from decoder_segment_ids) is compiled into
the instruction stream; the kernel is rebuilt per call.

On-device pipeline per (batch, segment, head, t-chunk of <=512 cols):
  - windowed fp32r QK matmuls (contraction over d on partitions) into PSUM
    group slots; segment sparsity = only the segment's s-blocks are computed,
    each with a column window [start_k, W)
  - rank-1 bf16 matmuls prefill window columns not covered by QK with -1e30
  - one grouped ACT Exp per 2 slots (PSUM -> SBUF, float32r out)
  - batched GPSIMD affine_select zeroes the causal triangles (exp -> 0)
  - fp32r PV matmuls accumulate out^T[d, t] in PSUM bank 1 of a 2-bank tile;
    M=1 ones-matmuls accumulate the softmax denominator row in bank 2
  - one DVE copy drains both banks; DMA to DRAM in transposed layout
Host side: Q/K are passed pre-transposed [.., d, t]; outputs are normalized
by the denominator and un-transposed in numpy.
"""

import math

import numpy as np

B, T, NQ, D = 2, 2048, 32, 128
NKV = 8
GH = NQ // NKV  # q heads per kv head = 4
NCORES = 8
CHUNK = 512  # max t-chunk width
SLOT = CHUNK + 128  # exp slot width (pad for triangle-select overrun)
NEG = -1.0e30


def _plan_batch(seg_row):
    """Per-batch plan: list of segments:
    (p0, L, chunks); chunk = (t0_rel, W, blocks);
    block = (k, rows, start, c0, diag) with start/c0 chunk-relative."""
    seg_row = np.asarray(seg_row)
    segs = []
    bounds = np.flatnonzero(np.diff(seg_row)) + 1
    edges = [0] + [int(x) for x in bounds] + [len(seg_row)]
    for p0, p1 in zip(edges[:-1], edges[1:]):
        if p1 <= p0:
            continue
        L = p1 - p0
        chunks = []
        t0 = 0
        while t0 < L:
            W = min(CHUNK, L - t0)
            t1 = t0 + W
            blocks = []
            for k in range(math.ceil(t1 / 128)):
                rows = min(128, L - 128 * k)
                c0 = 128 * k - t0  # triangle col offset, multiple of 128
                start = max(0, min(c0, W - 256))
                if (W - start) % 2 == 1 and start > 0:
                    start -= 1  # fp32r needs an even moving free dim
                blocks.append((k, rows, start, c0, c0 >= 0))
            chunks.append((t0, W, blocks))
            t0 = t1
        segs.append((p0, L, chunks))
    return segs


def _build_program(plans):
    import concourse.mybir as mybir
    import concourse.tile as tile
    from concourse import bacc
    from concourse.bass_types import AP as RawAP

    f32 = mybir.dt.float32
    f32r = mybir.dt.float32r
    bf16 = mybir.dt.bfloat16
    u32 = mybir.dt.uint32

    nc = bacc.Bacc("TRN2", target_bir_lowering=False, debug=False)

    qT = nc.dram_tensor("qT", [B, GH, D, T], f32r, kind="ExternalInput").ap()
    kT = nc.dram_tensor("kT", [B, D, T], f32r, kind="ExternalInput").ap()
    vv = nc.dram_tensor("vv", [B, T, D], f32r, kind="ExternalInput").ap()
    ones_d = nc.dram_tensor("ones_d", [128, 1], f32r, kind="ExternalInput").ap()
    prel_d = nc.dram_tensor("prel_d", [1, 128], bf16, kind="ExternalInput").ap()
    prer_d = nc.dram_tensor("prer_d", [1, CHUNK], bf16, kind="ExternalInput").ap()
    outT = nc.dram_tensor("outT", [B, GH, D, T], f32, kind="ExternalOutput").ap()
    den = nc.dram_tensor("den", [B, GH, T], f32, kind="ExternalOutput").ap()

    def mm(out_ap, lhsT, rhs, n, start, stop, f32_fallback):
        """fp32r matmul; odd/small N uses plain fp32 (same PE cost there)."""
        if f32_fallback:
            lhsT = lhsT.bitcast(f32)
            rhs = rhs.bitcast(f32)
        nc.tensor.matmul(out_ap, lhsT, rhs, start=start, stop=stop)

    with tile.TileContext(nc) as tc:
        with (
            tc.tile_pool(name="const", bufs=1) as cpool,
            tc.tile_pool(name="kv", bufs=2) as kvpool,
            tc.tile_pool(name="vseg", bufs=2) as vpool,
            tc.tile_pool(name="q", bufs=3) as qpool,
            tc.tile_pool(name="ebuf", bufs=2) as epool,
            tc.tile_pool(name="ostage", bufs=3) as opool,
            tc.tile_pool(name="qk", bufs=2, space="PSUM") as qkpool,
            tc.tile_pool(name="outp", bufs=2, space="PSUM") as outpool,
        ):
            ones_r = cpool.tile([128, 1], f32r, tag="ones_r")
            nc.sync.dma_start(ones_r[:], ones_d)
            pre_l = cpool.tile([1, 128], bf16, tag="pre_l")
            nc.sync.dma_start(pre_l[:], prel_d)
            pre_r = cpool.tile([1, CHUNK], bf16, tag="pre_r")
            nc.sync.dma_start(pre_r[:], prer_d)

            for b in range(B):
                kt = kvpool.tile([D, T], f32r, tag="kt")
                nc.sync.dma_start(kt[:], kT[b])

                for p0, L, chunks in plans[b]:
                    nslot = math.ceil(L / 128)
                    nfull = L // 128
                    vt = vpool.tile([128, nslot, D], f32r, tag="vt")
                    if nfull:
                        nc.sync.dma_start(
                            vt[:, 0:nfull, :],
                            vv[b, p0 : p0 + 128 * nfull, :].rearrange(
                                "(k p) d -> p k d", p=128
                            ),
                        )
                    rem = L - 128 * nfull
                    if rem:
                        nc.sync.dma_start(
                            vt[0:rem, nfull, :],
                            vv[b, p0 + 128 * nfull : p0 + L, :],
                        )

                    for h in range(GH):
                        for t0, W, blocks in chunks:
                            a0 = p0 + t0  # absolute t start
                            qt = qpool.tile([D, CHUNK], f32r, tag="qt")
                            nc.sync.dma_start(qt[:, 0:W], qT[b, h, :, a0 : a0 + W])

                            n_s = len(blocks)
                            ebuf = epool.tile([128, n_s, SLOT], f32r, tag="ebuf")

                            # QK + exp in groups of 2 slots
                            for g0 in range(0, n_s, 2):
                                blks = blocks[g0 : g0 + 2]
                                glen = len(blks)
                                grp = qkpool.tile([128, 2, CHUNK], f32, tag="grp")
                                gstart = blks[0][2]
                                for j, (k, rows, start, c0, diag) in enumerate(blks):
                                    if start > gstart:
                                        nc.tensor.matmul(
                                            grp[:, j, gstart:start],
                                            pre_l[:, :],
                                            pre_r[:, 0 : start - gstart],
                                            start=True,
                                            stop=True,
                                        )
                                    s_abs = p0 + 128 * k
                                    mm(
                                        grp[0:rows, j, start:W],
                                        kt[:, s_abs : s_abs + rows],
                                        qt[:, start:W],
                                        W - start,
                                        True,
                                        True,
                                        (W - start) % 2 == 1,
                                    )
                                nc.scalar.activation(
                                    ebuf[:, g0 : g0 + glen, gstart:W],
                                    grp[:, 0:glen, gstart:W],
                                    mybir.ActivationFunctionType.Exp,
                                )

                            # zero pad regions [start, c0) of left-padded blocks
                            for si, (k, rows, start, c0, diag) in enumerate(blocks):
                                if diag and start < c0:
                                    nc.vector.memset(
                                        ebuf[:, si, start:c0].bitcast(u32), 0
                                    )

                            # batched causal triangle select over diag slots
                            dlist = [
                                (si, blk[3])
                                for si, blk in enumerate(blocks)
                                if blk[4]
                            ]
                            if dlist:
                                first_si, first_c0 = dlist[0]
                                n_diag = len(dlist)
                                e0 = ebuf[:]
                                src = RawAP(
                                    tensor=e0.tensor,
                                    offset=e0.offset + first_si * SLOT + first_c0,
                                    ap=[
                                        list(e0.ap[0]),
                                        [SLOT + 128, n_diag],
                                        [1, 128],
                                    ],
                                )
                                nc.gpsimd.affine_select(
                                    out=src,
                                    in_=src,
                                    fill=0.0,
                                    compare_op=mybir.AluOpType.is_ge,
                                    base=0,
                                    channel_multiplier=-1,
                                    pattern=[[0, n_diag], [1, 128]],
                                )

                            # PV (bank 1) + denominator (bank 2) accumulation
                            outp = outpool.tile([128, 2 * CHUNK], f32, tag="outp")
                            for si, (k, rows, start, c0, diag) in enumerate(blocks):
                                odd = (W - start) % 2 == 1
                                mm(
                                    outp[:, start:W],
                                    vt[0:rows, k, :],
                                    ebuf[0:rows, si, start:W],
                                    W - start,
                                    (si == 0),
                                    (si == n_s - 1),
                                    odd,
                                )
                                mm(
                                    outp[0:1, CHUNK + start : CHUNK + W],
                                    ones_r[0:rows, :],
                                    ebuf[0:rows, si, start:W],
                                    W - start,
                                    (si == 0),
                                    (si == n_s - 1),
                                    odd,
                                )

                            osb = opool.tile([128, 2 * CHUNK], f32, tag="osb")
                            nc.vector.tensor_copy(
                                osb[:, 0 : CHUNK + W], outp[:, 0 : CHUNK + W]
                            )
                            nc.sync.dma_start(
                                outT[b, h, :, a0 : a0 + W], osb[:, 0:W]
                            )
                            nc.sync.dma_start(
                                den[b, h, a0 : a0 + W],
                                osb[0:1, CHUNK : CHUNK + W],
                            )
    nc.compile()
    return nc


def _shard_inputs(query, key, value):
    qT = query.transpose(0, 2, 3, 1)  # [B, NQ, D, T]
    kTfull = key.transpose(0, 2, 3, 1)  # [B, NKV, D, T]
    ones = np.ones((128, 1), dtype=np.float32)
    import ml_dtypes

    prel = np.ones((1, 128), dtype=ml_dtypes.bfloat16)
    prer = np.full((1, CHUNK), NEG, dtype=ml_dtypes.bfloat16)
    in_maps = []
    for i in range(NCORES):
        in_maps.append(
            {
                "qT": np.ascontiguousarray(qT[:, GH * i : GH * (i + 1)]),
                "kT": np.ascontiguousarray(kTfull[:, i]),
                "vv": np.ascontiguousarray(value[:, :, i, :]),
                "ones_d": ones,
                "prel_d": prel,
                "prer_d": prer,
            }
        )
    return in_maps


def kernel(query, key, value, decoder_segment_ids):
    from concourse.bass_utils import run_bass_kernel_spmd

    query = np.ascontiguousarray(query, dtype=np.float32)
    key = np.ascontiguousarray(key, dtype=np.float32)
    value = np.ascontiguousarray(value, dtype=np.float32)
    seg = np.asarray(decoder_segment_ids)

    plans = [_plan_batch(seg[b]) for b in range(B)]
    nc = _build_program(plans)
    in_maps = _shard_inputs(query, key, value)

    res = run_bass_kernel_spmd(nc, in_maps, core_ids=list(range(NCORES)))

    out = np.empty((B, T, NQ, D), dtype=np.float32)
    for i in range(NCORES):
        oT = res.results[i]["outT"]  # [B, GH, D, T]
        dn = res.results[i]["den"]  # [B, GH, T]
        o = oT / dn[:, :, None, :]
        out[:, :, GH * i : GH * (i + 1), :] = o.transpose(0, 3, 1, 2)
    return out


# revision 6
# speedup vs baseline: 1.5509x; 1.5509x over previous
"""Trainium2 Bass kernel for GQA causal attention with packed-sequence segment
masking (B=2, T=2048, 32 q-heads, 8 kv-heads, D=128, fp32).

Sharding: 8 NeuronCores; core i handles kv-head i (and its 4 GQA query heads)
for both batch rows. One SPMD program; per-core data differs only in which
heads it gets. The segment layout (from decoder_segment_ids) is compiled into
the instruction stream; the kernel is rebuilt per call.

On-device pipeline per (batch, segment, head, t-chunk of <=512 cols):
  - windowed fp32r QK matmuls (contraction over d on partitions) into PSUM
    group slots; segment sparsity = only the segment's s-blocks are computed,
    each with a column window [start_k, W)
  - rank-1 bf16 matmuls prefill window columns not covered by QK with -1e30
  - one grouped ACT Exp per 2 slots (PSUM -> SBUF, float32r out)
  - batched GPSIMD affine_select zeroes the causal triangles (exp -> 0)
  - fp32r PV matmuls accumulate out^T[d, t] in PSUM bank 1 of a 2-bank tile;
    M=1 ones-matmuls accumulate the softmax denominator row in bank 2
  - one DVE copy drains both banks; DMA to DRAM in transposed layout
Host side: Q/K are passed pre-transposed [.., d, t]; outputs are normalized
by the denominator and un-transposed in numpy.
"""

import math

import numpy as np

B, T, NQ, D = 2, 2048, 32, 128
NKV = 8
GH = NQ // NKV  # q heads per kv head = 4
NCORES = 8
CHUNK = 512  # max t-chunk width
SLOT = CHUNK + 128  # exp slot width (pad for triangle-select overrun)
NEG = -1.0e30


def _plan_batch(seg_row):
    """Per-batch plan: list of segments:
    (p0, L, chunks); chunk = (t0_rel, W, blocks);
    block = (k, rows, start, c0, diag) with start/c0 chunk-relative."""
    seg_row = np.asarray(seg_row)
    segs = []
    bounds = np.flatnonzero(np.diff(seg_row)) + 1
    edges = [0] + [int(x) for x in bounds] + [len(seg_row)]
    for p0, p1 in zip(edges[:-1], edges[1:]):
        if p1 <= p0:
            continue
        L = p1 - p0
        chunks = []
        t0 = 0
        while t0 < L:
            W = min(CHUNK, L - t0)
            t1 = t0 + W
            blocks = []
            for k in range(math.ceil(t1 / 128)):
                rows = min(128, L - 128 * k)
                c0 = 128 * k - t0  # triangle col offset, multiple of 128
                start = max(0, min(c0, W - 256))
                if (W - start) % 2 == 1 and start > 0:
                    start -= 1  # fp32r needs an even moving free dim
                blocks.append((k, rows, start, c0, c0 >= 0))
            chunks.append((t0, W, blocks))
            t0 = t1
        segs.append((p0, L, chunks))
    return segs


def _build_program(plans, repeat=1):
    import concourse.mybir as mybir
    import concourse.tile as tile
    from concourse import bacc
    from concourse.bass_types import AP as RawAP

    f32 = mybir.dt.float32
    f32r = mybir.dt.float32r
    bf16 = mybir.dt.bfloat16
    u32 = mybir.dt.uint32

    nc = bacc.Bacc("TRN2", target_bir_lowering=False, debug=False)

    qT = nc.dram_tensor("qT", [B, GH, D, T], f32r, kind="ExternalInput").ap()
    kT = nc.dram_tensor("kT", [B, D, T], f32r, kind="ExternalInput").ap()
    vv = nc.dram_tensor("vv", [B, T, D], f32r, kind="ExternalInput").ap()
    ones_d = nc.dram_tensor("ones_d", [128, 1], f32r, kind="ExternalInput").ap()
    prel_d = nc.dram_tensor("prel_d", [1, 128], bf16, kind="ExternalInput").ap()
    prer_d = nc.dram_tensor("prer_d", [1, CHUNK], bf16, kind="ExternalInput").ap()
    outT = nc.dram_tensor("outT", [B, GH, D, T], f32, kind="ExternalOutput").ap()
    den = nc.dram_tensor("den", [B, GH, T], f32, kind="ExternalOutput").ap()

    def mm(out_ap, lhsT, rhs, n, start, stop, f32_fallback):
        """fp32r matmul; odd/small N uses plain fp32 (same PE cost there)."""
        if f32_fallback:
            lhsT = lhsT.bitcast(f32)
            rhs = rhs.bitcast(f32)
        nc.tensor.matmul(out_ap, lhsT, rhs, start=start, stop=stop)

    with tile.TileContext(nc) as tc:
        with (
            tc.tile_pool(name="const", bufs=1) as cpool,
            tc.tile_pool(name="kv", bufs=2) as kvpool,
            tc.tile_pool(name="vseg", bufs=2) as vpool,
            tc.tile_pool(name="q", bufs=3) as qpool,
            tc.tile_pool(name="ebuf", bufs=2) as epool,
            tc.tile_pool(name="ostage", bufs=3) as opool,
            tc.tile_pool(name="qk", bufs=2, space="PSUM") as qkpool,
            tc.tile_pool(name="outp", bufs=2, space="PSUM") as outpool,
        ):
            ones_r = cpool.tile([128, 1], f32r, tag="ones_r")
            nc.sync.dma_start(ones_r[:], ones_d)
            pre_l = cpool.tile([1, 128], bf16, tag="pre_l")
            nc.sync.dma_start(pre_l[:], prel_d)
            pre_r = cpool.tile([1, CHUNK], bf16, tag="pre_r")
            nc.sync.dma_start(pre_r[:], prer_d)

            for _rep, b in [(r, bb) for r in range(repeat) for bb in range(B)]:
                kt = kvpool.tile([D, T], f32r, tag="kt")
                nc.sync.dma_start(kt[:], kT[b])

                for p0, L, chunks in plans[b]:
                    nslot = math.ceil(L / 128)
                    nfull = L // 128
                    vt = vpool.tile([128, nslot, D], f32r, tag="vt")
                    if nfull:
                        nc.sync.dma_start(
                            vt[:, 0:nfull, :],
                            vv[b, p0 : p0 + 128 * nfull, :].rearrange(
                                "(k p) d -> p k d", p=128
                            ),
                        )
                    rem = L - 128 * nfull
                    if rem:
                        nc.sync.dma_start(
                            vt[0:rem, nfull, :],
                            vv[b, p0 + 128 * nfull : p0 + L, :],
                        )

                    for h in range(GH):
                        for t0, W, blocks in chunks:
                            a0 = p0 + t0  # absolute t start
                            qt = qpool.tile([D, CHUNK], f32r, tag="qt")
                            nc.sync.dma_start(qt[:, 0:W], qT[b, h, :, a0 : a0 + W])

                            n_s = len(blocks)
                            ebuf = epool.tile([128, n_s, SLOT], f32r, tag="ebuf")

                            # QK + exp in groups of 2 slots
                            for g0 in range(0, n_s, 2):
                                blks = blocks[g0 : g0 + 2]
                                glen = len(blks)
                                grp = qkpool.tile([128, 2, CHUNK], f32, tag="grp")
                                gstart = blks[0][2]
                                for j, (k, rows, start, c0, diag) in enumerate(blks):
                                    if start > gstart:
                                        nc.tensor.matmul(
                                            grp[:, j, gstart:start],
                                            pre_l[:, :],
                                            pre_r[:, 0 : start - gstart],
                                            start=True,
                                            stop=True,
                                        )
                                    s_abs = p0 + 128 * k
                                    mm(
                                        grp[0:rows, j, start:W],
                                        kt[:, s_abs : s_abs + rows],
                                        qt[:, start:W],
                                        W - start,
                                        True,
                                        True,
                                        (W - start) % 2 == 1,
                                    )
                                nc.scalar.activation(
                                    ebuf[:, g0 : g0 + glen, gstart:W],
                                    grp[:, 0:glen, gstart:W],
                                    mybir.ActivationFunctionType.Exp,
                                )

                            # zero pad regions [start, c0) of left-padded blocks
                            for si, (k, rows, start, c0, diag) in enumerate(blocks):
                                if diag and start < c0:
                                    nc.vector.memset(
                                        ebuf[:, si, start:c0].bitcast(u32), 0
                                    )

                            # batched causal triangle select over diag slots
                            dlist = [
                                (si, blk[3])
                                for si, blk in enumerate(blocks)
                                if blk[4]
                            ]
                            if dlist:
                                first_si, first_c0 = dlist[0]
                                n_diag = len(dlist)
                                e0 = ebuf[:]
                                src = RawAP(
                                    tensor=e0.tensor,
                                    offset=e0.offset + first_si * SLOT + first_c0,
                                    ap=[
                                        list(e0.ap[0]),
                                        [SLOT + 128, n_diag],
                                        [1, 128],
                                    ],
                                )
                                nc.gpsimd.affine_select(
                                    out=src,
                                    in_=src,
                                    fill=0.0,
                                    compare_op=mybir.AluOpType.is_ge,
                                    base=0,
                                    channel_multiplier=-1,
                                    pattern=[[0, n_diag], [1, 128]],
                                )

                            # PV (bank 1) + denominator (bank 2) accumulation
                            outp = outpool.tile([128, 2 * CHUNK], f32, tag="outp")
                            for si, (k, rows, start, c0, diag) in enumerate(blocks):
                                odd = (W - start) % 2 == 1
                                mm(
                                    outp[:, start:W],
                                    vt[0:rows, k, :],
                                    ebuf[0:rows, si, start:W],
                                    W - start,
                                    (si == 0),
                                    (si == n_s - 1),
                                    odd,
                                )
                                mm(
                                    outp[0:1, CHUNK + start : CHUNK + W],
                                    ones_r[0:rows, :],
                                    ebuf[0:rows, si, start:W],
                                    W - start,
                                    (si == 0),
                                    (si == n_s - 1),
                                    odd,
                                )

                            osb = opool.tile([128, 2 * CHUNK], f32, tag="osb")
                            nc.vector.tensor_copy(
                                osb[:, 0 : CHUNK + W], outp[:, 0 : CHUNK + W]
                            )
                            nc.sync.dma_start(
                                outT[b, h, :, a0 : a0 + W], osb[:, 0:W]
                            )
                            nc.sync.dma_start(
                                den[b, h, a0 : a0 + W],
                                osb[0:1, CHUNK : CHUNK + W],
                            )
    nc.compile()
    return nc


def _shard_inputs(query, key, value):
    qT = query.transpose(0, 2, 3, 1)  # [B, NQ, D, T]
    kTfull = key.transpose(0, 2, 3, 1)  # [B, NKV, D, T]
    ones = np.ones((128, 1), dtype=np.float32)
    import ml_dtypes

    prel = np.ones((1, 128), dtype=ml_dtypes.bfloat16)
    prer = np.full((1, CHUNK), NEG, dtype=ml_dtypes.bfloat16)
    in_maps = []
    for i in range(NCORES):
        in_maps.append(
            {
                "qT": np.ascontiguousarray(qT[:, GH * i : GH * (i + 1)]),
                "kT": np.ascontiguousarray(kTfull[:, i]),
                "vv": np.ascontiguousarray(value[:, :, i, :]),
                "ones_d": ones,
                "prel_d": prel,
                "prer_d": prer,
            }
        )
    return in_maps


def kernel(query, key, value, decoder_segment_ids):
    from concourse.bass_utils import run_bass_kernel_spmd

    query = np.ascontiguousarray(query, dtype=np.float32)
    key = np.ascontiguousarray(key, dtype=np.float32)
    value = np.ascontiguousarray(value, dtype=np.float32)
    seg = np.asarray(decoder_segment_ids)

    plans = [_plan_batch(seg[b]) for b in range(B)]
    nc = _build_program(plans)
    in_maps = _shard_inputs(query, key, value)

    res = run_bass_kernel_spmd(nc, in_maps, core_ids=list(range(NCORES)))

    out = np.empty((B, T, NQ, D), dtype=np.float32)
    for i in range(NCORES):
        oT = res.results[i]["outT"]  # [B, GH, D, T]
        dn = res.results[i]["den"]  # [B, GH, T]
        o = oT / dn[:, :, None, :]
        out[:, :, GH * i : GH * (i + 1), :] = o.transpose(0, 3, 1, 2)
    return out
